# revision 1
# baseline (speedup 1.0000x reference)
# Trainium2 Bass kernel for nn_Net_4861902979707
#
# Computation (per sample, B = 4194304):
#   X [B, 3, 3] -> 3 pairwise Euclidean distances d = [d01, d02, d12]
#   h1 = elu(d @ W1.T + b1); h2 = elu(h1 @ W2.T + b2); y = h2 @ W3.T + b3
#
# Strategy: pure data parallel over 8 NeuronCores (batch split), sample-major
# layout on chip: tiles of [128 partitions, T samples]. Distances partly on
# DVE (pairwise diffs as contiguous-write "plane" ops; d12 = d02 - d01),
# squares on DVE/ACT, then the TensorEngine does every linear reduction as
# diagonal-matrix matmuls accumulated in PSUM fp32: coord sums (identity
# lhsT), all three MLP layers (W*I lhsT). ELU is elu(z)+1 = relu(z+b) +
# exp(-relu(-(z+b))) on ACT (biases fused into the activation); the +1
# shift is absorbed into the next layer's bias on the host (b' = b - W @ 1).
import os as _os
import numpy as np

B = 4194304
N_CORES = 8
B_CORE = B // N_CORES          # 524288
P = 128                        # partitions
T = int(_os.environ.get("TSZ", "512"))
TILE = P * T
N_TILES = B_CORE // TILE

# intermediate dtype: "bf16" (fast) or "fp32" (accurate)
COMPUTE_DT = "bf16"

XCAST = _os.environ.get("XCAST", "0") == "1"    # X delivered as bf16 (host cast)
SQ_ACT = int(_os.environ.get("SQ_ACT", "0"))    # pairs squared on ACT (0-3)
ELU_MODE = _os.environ.get("ELU_MODE", "dve")   # act | dve
STAGE = _os.environ.get("STAGE", "full")        # full | dma | dist
BUFS_X = int(_os.environ.get("BUFS_X", "4"))
BUFS_W = int(_os.environ.get("BUFS_W", "3"))
BUFS_M = int(_os.environ.get("BUFS_M", "3"))

_CACHE = {}


def _split_sync_waits(nc, mybir, limit=1):
    """This walrus build rejects instructions carrying more than ~1 sem wait
    ("Too many sync wait commands"). Hoist excess waits onto NoOp carrier
    instructions (same engine, immediately before) — engine program order
    preserves the blocking semantics."""
    n_split = 0
    for f in nc.m.functions:
        for b in f.blocks:
            lst = b.instructions
            out = []
            changed = False
            for inst in lst:
                si = inst.sync_info
                if si is not None and len(si.on_wait) > limit:
                    waits = list(si.on_wait)
                    extra, keep = waits[:-limit], waits[-limit:]
                    for wi, w in enumerate(extra):
                        nop = mybir.InstNoOp(
                            name=f"wsplit-{inst.name}-{wi}")
                        nop.engine = inst.engine
                        nop.sync_info = mybir.SyncInfo(
                            on_wait=[w], on_update=[])
                        out.append(nop)
                        n_split += 1
                    inst.sync_info = type(si)(
                        on_wait=keep, on_update=list(si.on_update))
                    changed = True
                out.append(inst)
            if changed:
                b.instructions = out
    return n_split


# WD diag-matrix indices (each a [128,128] lhsT); weights stored as
# bf16 hi+lo pairs so the PE path keeps ~fp32 weight precision
def _iWD_I():
    return 0
def _iWD_W1(k, j, p):
    return 1 + 2 * (3 * k + j) + p
def _iWD_W2(m, j, p):
    return 13 + 2 * (2 * m + j) + p
def _iWD_W3(j, p):
    return 21 + 2 * j + p
N_WD = 25

# WB scalar indices: b1[k]=k, b2'[m]=2+m, b3'=4, -b1[k]=5+k, -b2'[m]=7+m
def _ib1(k):
    return k
def _ib2(m):
    return 2 + m
_IB3 = 4
def _inb1(k):
    return 5 + k
def _inb2(m):
    return 7 + m
N_WB = 9


def _build(dt_name, reps=1, bench_small=False):
    import concourse.bass as bass
    import concourse.tile as tile
    import concourse.mybir as mybir

    f32 = mybir.dt.float32
    bf16 = mybir.dt.bfloat16
    dt = bf16 if dt_name == "bf16" else f32
    Alu = mybir.AluOpType
    Act = mybir.ActivationFunctionType

    nc = bass.Bass()
    BC = TILE if bench_small else B_CORE
    X = nc.dram_tensor("X", [BC, 9], dt if XCAST else f32,
                       kind="ExternalInput")
    WB = nc.dram_tensor("WB", [N_WB], f32, kind="ExternalInput")
    WD = nc.dram_tensor("WD", [N_WD, P, P], dt, kind="ExternalInput")
    Y = nc.dram_tensor("Y", [BC, 1], f32, kind="ExternalOutput")

    PAIRS = [(0, 1), (0, 2)]  # pair 2 (1,2) comes from d02 - d01

    with tile.TileContext(nc) as tc:
        with (
            tc.tile_pool(name="singles", bufs=1) as singles,
            tc.tile_pool(name="xin", bufs=BUFS_X) as xin,
            tc.tile_pool(name="work", bufs=BUFS_W) as work,
            tc.tile_pool(name="mlp", bufs=BUFS_M) as mlp,
            tc.tile_pool(name="yout", bufs=3) as yout,
            tc.tile_pool(name="psum", bufs=1, space="PSUM") as psum,
        ):
            # broadcast bias scalars to all partitions; load diag matrices
            wb = singles.tile([P, N_WB], f32)
            nc.gpsimd.dma_start(
                out=wb[:],
                in_=bass.AP(tensor=WB[:].tensor, offset=0,
                            ap=[[0, P], [1, N_WB]]))
            wd = singles.tile([P, N_WD, P], dt)
            nc.sync.dma_start(
                out=wd[:],
                in_=bass.AP(tensor=WD[:].tensor, offset=0,
                            ap=[[P, P], [P * P, N_WD], [1, P]]))

            def ws(i):  # [P,1] bias scalar AP
                return wb[:, i:i + 1]

            def diag(i):  # [128,128] lhsT AP
                return wd[:, i, :]

            # reps>1 wraps the whole body in a For_i loop (benchmarking only)
            _loop = tc.For_i(0, reps) if reps != 1 else None
            if _loop is not None:
                _loop.__enter__()

            for ti in range(N_TILES):
                src = 0 if bench_small else ti
                xr = X[src * TILE:(src + 1) * TILE, :].rearrange(
                    "(p s) d -> p s d", p=P)
                xt = xin.tile([P, T, 9], dt if XCAST else f32)
                nc.sync.dma_start(out=xt[:], in_=xr)

                yr = Y[src * TILE:(src + 1) * TILE, :].rearrange(
                    "(p s) d -> p (s d)", p=P)

                if STAGE == "dma":
                    yt = yout.tile([P, T], f32)
                    nc.scalar.activation(yt, xt[:, :, 0], Act.Copy)
                    nc.sync.dma_start(out=yr, in_=yt[:])
                    continue

                # pairwise diffs -> planes [P, 9, T]; contiguous writes
                diff = work.tile([P, 9, T], dt)
                for pi, (i, j) in enumerate(PAIRS):
                    for c in range(3):
                        nc.vector.tensor_sub(
                            diff[:, 3 * pi + c, :],
                            xt[:, :, 3 * i + c],
                            xt[:, :, 3 * j + c],
                        )
                # d12 = d02 - d01 (contiguous bf16, 2x)
                nc.vector.tensor_sub(
                    diff[:, 6:9, :], diff[:, 3:6, :], diff[:, 0:3, :])

                # squares in place, per pair (SQ_ACT of them on ACT)
                sq = diff
                for pi in range(3):
                    pl = diff[:, 3 * pi:3 * pi + 3, :]
                    if pi < SQ_ACT:
                        nc.scalar.activation(pl, pl, Act.Square)
                    else:
                        nc.vector.tensor_mul(pl, pl, pl)

                # coord sums on PE: q_pi = I@sq0 + I@sq1 + I@sq2 (PSUM fp32)
                qs = []
                for pi in range(3):
                    q = psum.tile([P, T], f32, tag=f"q{pi}")
                    for c in range(3):
                        nc.tensor.matmul(
                            q[:], diag(_iWD_I()), sq[:, 3 * pi + c, :],
                            start=(c == 0), stop=(c == 2))
                    qs.append(q)

                # distances (ACT sqrt, PSUM -> SBUF bf16)
                dist = work.tile([P, 3, T], dt)
                for pi in range(3):
                    nc.scalar.activation(dist[:, pi, :], qs[pi][:], Act.Sqrt)

                if STAGE == "dist":
                    yt = yout.tile([P, T], f32)
                    nc.scalar.activation(yt, dist[:, 0, :], Act.Copy)
                    nc.sync.dma_start(out=yr, in_=yt[:])
                    continue

                def elu(z_psum, ib, inb, tag):
                    """h = relu(z+b) + exp(min(z+b, 0)) from PSUM z."""
                    r = mlp.tile([P, T], dt, tag=f"r_{tag}")
                    nc.scalar.activation(
                        r, z_psum[:], Act.Relu, bias=ws(ib), scale=1.0)
                    e = mlp.tile([P, T], dt, tag=f"e_{tag}")
                    if ELU_MODE == "act":
                        rm = mlp.tile([P, T], dt, tag=f"rm_{tag}")
                        nc.scalar.activation(
                            rm, z_psum[:], Act.Relu, bias=ws(inb), scale=-1.0)
                        nc.scalar.activation(e, rm, Act.Exp, scale=-1.0)
                    else:
                        m = mlp.tile([P, T], dt, tag=f"rm_{tag}")
                        nc.vector.tensor_scalar(
                            out=m, in0=z_psum[:], scalar1=ws(ib),
                            scalar2=0.0, op0=Alu.add, op1=Alu.min)
                        nc.scalar.activation(e, m, Act.Exp)
                    h = mlp.tile([P, T], dt, tag=f"h_{tag}")
                    nc.vector.tensor_add(h, r, e)
                    return h

                # L1 on PE: z_k = sum_j W1[k,j]*I @ d_j  (PSUM fp32)
                h1 = []
                for k in range(2):
                    z = psum.tile([P, T], f32, tag=f"z1_{k}")
                    for j in range(3):
                        for p_ in range(2):
                            nc.tensor.matmul(
                                z[:], diag(_iWD_W1(k, j, p_)), dist[:, j, :],
                                start=(j == 0 and p_ == 0),
                                stop=(j == 2 and p_ == 1))
                    h1.append(elu(z, _ib1(k), _inb1(k), f"1{k}"))

                # L2
                h2 = []
                for m_ in range(2):
                    z = psum.tile([P, T], f32, tag=f"z2_{m_}")
                    for j in range(2):
                        for p_ in range(2):
                            nc.tensor.matmul(
                                z[:], diag(_iWD_W2(m_, j, p_)), h1[j][:],
                                start=(j == 0 and p_ == 0),
                                stop=(j == 1 and p_ == 1))
                    h2.append(elu(z, _ib2(m_), _inb2(m_), f"2{m_}"))

                # L3
                yz = psum.tile([P, T], f32, tag="yz")
                for j in range(2):
                    for p_ in range(2):
                        nc.tensor.matmul(
                            yz[:], diag(_iWD_W3(j, p_)), h2[j][:],
                            start=(j == 0 and p_ == 0),
                            stop=(j == 1 and p_ == 1))
                yt = yout.tile([P, T], f32)
                nc.scalar.activation(
                    yt, yz[:], Act.Identity, bias=ws(_IB3), scale=1.0)
                nc.sync.dma_start(out=yr, in_=yt[:])

            if _loop is not None:
                _loop.__exit__(None, None, None)

    _split_sync_waits(nc, mybir, limit=1)
    return nc


def _pack_weights(W1, b1, W2, b2, W3, b3):
    import ml_dtypes
    W1 = np.asarray(W1, np.float32); b1 = np.asarray(b1, np.float32)
    W2 = np.asarray(W2, np.float32); b2 = np.asarray(b2, np.float32)
    W3 = np.asarray(W3, np.float32); b3 = np.asarray(b3, np.float32)
    wb = np.empty(N_WB, np.float32)
    b2a = b2 - W2.sum(axis=1)            # absorb elu(+1) shift
    b3a = b3 - W3.sum(axis=1)
    wb[0:2] = b1
    wb[2:4] = b2a
    wb[4] = b3a[0]
    wb[5:7] = -b1
    wb[7:9] = -b2a

    dt = ml_dtypes.bfloat16 if COMPUTE_DT == "bf16" else np.float32
    eye = np.eye(P, dtype=np.float32)

    def hilo(w):
        hi = np.float32(np.asarray(w, dt).astype(np.float32))
        lo = np.float32(w) - hi
        return hi, lo

    wdf = np.empty((N_WD, P, P), np.float32)
    wdf[_iWD_I()] = eye
    for k in range(2):
        for j in range(3):
            hi, lo = hilo(W1[k, j])
            wdf[_iWD_W1(k, j, 0)] = eye * hi
            wdf[_iWD_W1(k, j, 1)] = eye * lo
    for m in range(2):
        for j in range(2):
            hi, lo = hilo(W2[m, j])
            wdf[_iWD_W2(m, j, 0)] = eye * hi
            wdf[_iWD_W2(m, j, 1)] = eye * lo
    for j in range(2):
        hi, lo = hilo(W3[0, j])
        wdf[_iWD_W3(j, 0)] = eye * hi
        wdf[_iWD_W3(j, 1)] = eye * lo
    return wb, wdf.astype(dt)


LAST_RESULTS = None  # BassKernelResults of the most recent run (for test.py)


def kernel(X, W1, b1, W2, b2, W3, b3):
    from concourse.bass_utils import run_bass_kernel_spmd
    import ml_dtypes
    global LAST_RESULTS

    X = np.ascontiguousarray(np.asarray(X, np.float32).reshape(B, 9))
    if XCAST:
        X = X.astype(ml_dtypes.bfloat16 if COMPUTE_DT == "bf16"
                     else np.float32)
    wb, wd = _pack_weights(W1, b1, W2, b2, W3, b3)

    key = (COMPUTE_DT, 1)
    if key not in _CACHE:
        _CACHE[key] = _build(COMPUTE_DT)
    nc = _CACHE[key]

    in_maps = [
        {"X": X[c * B_CORE:(c + 1) * B_CORE], "WB": wb, "WD": wd}
        for c in range(N_CORES)
    ]
    res = run_bass_kernel_spmd(nc, in_maps, core_ids=list(range(N_CORES)))
    LAST_RESULTS = res
    out = np.concatenate([res.results[c]["Y"] for c in range(N_CORES)], axis=0)
    return out.reshape(B, 1)



# revision 8
# speedup vs baseline: 1.1219x; 1.1219x over previous
# Trainium2 Bass kernel for nn_Net_4861902979707
#
# Computation (per sample, B = 4194304):
#   X [B, 3, 3] -> 3 pairwise Euclidean distances d = [d01, d02, d12]
#   h1 = elu(d @ W1.T + b1); h2 = elu(h1 @ W2.T + b2); y = h2 @ W3.T + b3
#
# Strategy: pure data parallel over 8 NeuronCores (batch split). Host does
# layout/dtype only: X is cast to fp16 and each 128xT tile is transposed to
# channel-major [128 partitions, 9 coord planes, T samples] so every on-chip
# op is a fat contiguous instruction.
#
# The per-tile computation is a long cross-engine chain (DVE diffs/squares ->
# PE coord sums -> ACT sqrt -> PE L1 -> ACT/DVE elu -> PE L2 -> ... -> DMA).
# Engines execute their instruction streams in order, so emitting tile-by-
# tile serializes the chain (the baseline ran at the SUM of engine times).
# This kernel instead emits a 6-deep software pipeline; at each step every
# engine's next instruction depends only on work from previous steps (or
# early same-step ops), so all engines run concurrently:
#   step i   : DMA in tile i
#   step i+1 : DVE diffs (3 fat subs) + squares of planes 0-5 (fat)
#   step i+2 : GP squares planes 6-8 + GP q12 sums; PE q01/q02 (identity
#              diag, PSUM); ACT sqrt(q01,q02)
#   step i+3 : ACT sqrt(q12); PE L1 (bias via ones-plane + diag passes);
#              ACT exp1/relu1 (fat, bias in PSUM); DVE h1 combine (fused STT)
#   step i+4 : PE L2 (bias + 4 diag passes)
#   step i+5 : ACT exp2/relu2 (fat); DVE h2 combine, L3 (TS + STT); DMA out
# ELU identity: elu(z)+1 = relu(z) + min(exp(z), 1); the +1 shift is
# absorbed into the next layer's bias on the host (b' = b - W @ 1).
# Everything on chip is fp16 (PSUM fp32), rel err ~1e-3.
import os as _os
import numpy as np

B = 4194304
N_CORES = 8
B_CORE = B // N_CORES          # 524288
P = 128                        # partitions
T = int(_os.environ.get("TSZ", "512"))
TILE = P * T
N_TILES = B_CORE // TILE

SQ_DVE = int(_os.environ.get("SQ_DVE", "6"))   # square planes on DVE (rest
SQ_ACT = int(_os.environ.get("SQ_ACT", "0"))   # on ACT, then GP)
RELU_ENG = _os.environ.get("RELU", "act")      # act | dve | mix
COMB = _os.environ.get("COMB", "std")          # std | split (L2 feed)
DEPTH = 6

COMPUTE_DT = "fp16"

_CACHE = {}


def _split_sync_waits(nc, mybir, limit=1):
    """This walrus build rejects instructions carrying more than ~1 sem wait
    ("Too many sync wait commands"). Hoist excess waits onto NoOp carrier
    instructions (same engine, immediately before) — engine program order
    preserves the blocking semantics."""
    n_split = 0
    for f in nc.m.functions:
        for b in f.blocks:
            lst = b.instructions
            out = []
            changed = False
            for inst in lst:
                si = inst.sync_info
                if si is not None and len(si.on_wait) > limit:
                    waits = list(si.on_wait)
                    extra, keep = waits[:-limit], waits[-limit:]
                    for wi, w in enumerate(extra):
                        nop = mybir.InstNoOp(
                            name=f"wsplit-{inst.name}-{wi}")
                        nop.engine = inst.engine
                        nop.sync_info = mybir.SyncInfo(
                            on_wait=[w], on_update=[])
                        out.append(nop)
                        n_split += 1
                    inst.sync_info = type(si)(
                        on_wait=keep, on_update=list(si.on_update))
                    changed = True
                out.append(inst)
            if changed:
                b.instructions = out
    return n_split


# WD diag-matrix indices (each a [128,128] fp16 lhsT)
def _iWD_I():
    return 0
def _iWD_W1(k, j):
    return 1 + 3 * k + j
def _iWD_W2(m, j):
    return 7 + 2 * m + j
def _iWD_B1(k):
    return 11 + k
def _iWD_B2(m):
    return 13 + m
N_WD = 15

# WB scalar indices
_IB3 = 4
_IW30 = 5
_IW31 = 6
N_WB = 7


def _build(dt_name=None, reps=1, bench_small=False):
    import concourse.bass as bass
    import concourse.tile as tile
    import concourse.mybir as mybir

    f32 = mybir.dt.float32
    f16 = mybir.dt.float16
    Alu = mybir.AluOpType
    Act = mybir.ActivationFunctionType

    nc = bass.Bass()
    NT = 1 if bench_small else N_TILES
    X = nc.dram_tensor("X", [NT * P, 9 * T], f16, kind="ExternalInput")
    WB = nc.dram_tensor("WB", [N_WB], f32, kind="ExternalInput")
    WD = nc.dram_tensor("WD", [N_WD, P, P], f16, kind="ExternalInput")
    Y = nc.dram_tensor("Y", [NT * P, T], f16, kind="ExternalOutput")

    with tile.TileContext(nc) as tc:
        with (
            tc.tile_pool(name="singles", bufs=1) as singles,
            tc.tile_pool(name="xin", bufs=4) as xin,
            tc.tile_pool(name="diffp", bufs=3) as diffp,
            tc.tile_pool(name="distp", bufs=3) as distp,
            tc.tile_pool(name="elup", bufs=2) as elup,
            tc.tile_pool(name="h1p", bufs=3) as h1p,
            tc.tile_pool(name="outp", bufs=3) as outp,
            tc.tile_pool(name="ps1", bufs=1, space="PSUM") as ps1,
            tc.tile_pool(name="ps2", bufs=2, space="PSUM") as ps2,
        ):
            # broadcast bias/weight scalars to all partitions; load diags
            wb = singles.tile([P, N_WB], f32)
            nc.gpsimd.dma_start(
                out=wb[:],
                in_=bass.AP(tensor=WB[:].tensor, offset=0,
                            ap=[[0, P], [1, N_WB]]))
            wd = singles.tile([P, N_WD, P], f16)
            nc.sync.dma_start(
                out=wd[:],
                in_=bass.AP(tensor=WD[:].tensor, offset=0,
                            ap=[[P, P], [P * P, N_WD], [1, P]]))
            ones = singles.tile([P, T], f16)
            nc.vector.memset(ones[:], 1.0)

            def ws(i):  # [P,1] scalar AP
                return wb[:, i:i + 1]

            def diag(i):  # [128,128] lhsT AP
                return wd[:, i, :]

            # per-tile state, indexed mod a small window
            WIN = DEPTH + 1
            st = [dict() for _ in range(WIN)]

            def S(i):
                return st[i % WIN]

            a0 = SQ_DVE
            a1 = SQ_DVE + SQ_ACT

            def step(s):
                i_dma = s
                i_s1 = s - 1
                i_s2 = s - 2
                i_b1 = s - 3
                i_b2 = s - 4
                i_b3 = s - 5

                def live(i):
                    return 0 <= i < N_TILES

                # --- DMA in (tile s) ---
                if live(i_dma):
                    src = 0 if bench_small else i_dma
                    xt = xin.tile([P, 9, T], f16)
                    nc.sync.dma_start(
                        out=xt[:], in_=X[src * P:(src + 1) * P, :])
                    S(i_dma)["xt"] = xt

                # --- B1 head: sqrt of q12 (frees PE L1 right after) ---
                if live(i_b1):
                    d = S(i_b1)
                    nc.scalar.activation(
                        d["dist"][:, 2, :], d["q12"], Act.Sqrt)

                # --- B1: L1 matmuls (bias plane + diag passes) ---
                if live(i_b1):
                    d = S(i_b1)
                    z1 = ps1.tile([P, 2, T], f32, tag="z1")
                    for k in range(2):
                        nc.tensor.matmul(
                            z1[:, k, :], diag(_iWD_B1(k)), ones[:],
                            start=True, stop=False)
                        for j in range(3):
                            nc.tensor.matmul(
                                z1[:, k, :], diag(_iWD_W1(k, j)),
                                d["dist"][:, j, :],
                                start=False, stop=(j == 2))
                    d["z1"] = z1

                # --- B3: exp2/relu2 (z2 from previous step; fat) ---
                if live(i_b3):
                    d = S(i_b3)
                    e2 = elup.tile([P, 2, T], f16, tag="e2")
                    r2 = elup.tile([P, 2, T], f16, tag="r2")
                    nc.scalar.activation(e2[:], d["z2"][:], Act.Exp)
                    if RELU_ENG in ("act", "mix"):
                        nc.scalar.activation(r2[:], d["z2"][:], Act.Relu)
                    else:
                        nc.vector.tensor_scalar(
                            out=r2[:], in0=d["z2"][:], scalar1=0.0,
                            scalar2=None, op0=Alu.max)
                    d["e2"], d["r2"] = e2, r2

                # --- S1: diffs + squares planes 0..a0 (DVE) ---
                if live(i_s1):
                    d = S(i_s1)
                    xt = d["xt"]
                    diff = diffp.tile([P, 9, T], f16)
                    nc.vector.tensor_sub(
                        diff[:, 0:3, :], xt[:, 0:3, :], xt[:, 3:6, :])
                    nc.vector.tensor_sub(
                        diff[:, 3:6, :], xt[:, 0:3, :], xt[:, 6:9, :])
                    nc.vector.tensor_sub(
                        diff[:, 6:9, :], diff[:, 3:6, :], diff[:, 0:3, :])
                    if a0:
                        nc.vector.tensor_mul(
                            diff[:, 0:a0, :], diff[:, 0:a0, :],
                            diff[:, 0:a0, :])
                    d["diff"] = diff

                # --- B3: h2 combine + L3 + DMA out (DVE) ---
                if live(i_b3):
                    d = S(i_b3)
                    h2 = outp.tile([P, 2, T], f16, tag="h2")
                    nc.vector.scalar_tensor_tensor(
                        out=h2[:], in0=d["e2"][:], scalar=1.0,
                        in1=d["r2"][:], op0=Alu.min, op1=Alu.add)
                    u = outp.tile([P, T], f16, tag="u")
                    nc.vector.tensor_scalar(
                        out=u, in0=h2[:, 0, :], scalar1=ws(_IW30),
                        scalar2=ws(_IB3), op0=Alu.mult, op1=Alu.add)
                    yt = outp.tile([P, T], f16, tag="yt")
                    nc.vector.scalar_tensor_tensor(
                        out=yt, in0=h2[:, 1, :], scalar=ws(_IW31), in1=u,
                        op0=Alu.mult, op1=Alu.add)
                    src = 0 if bench_small else i_b3
                    nc.sync.dma_start(
                        out=Y[src * P:(src + 1) * P, :], in_=yt[:])

                # --- S2: GP squares planes a1..8 + q12 sums ---
                if live(i_s2):
                    d = S(i_s2)
                    diff = d["diff"]
                    if a1 < 9:
                        nc.gpsimd.tensor_mul(
                            diff[:, a1:9, :], diff[:, a1:9, :],
                            diff[:, a1:9, :])
                    if SQ_ACT:
                        nc.scalar.activation(
                            diff[:, a0:a1, :], diff[:, a0:a1, :], Act.Square)
                    q12 = distp.tile([P, T], f16, tag="q12")
                    nc.gpsimd.tensor_add(q12, diff[:, 6, :], diff[:, 7, :])
                    nc.gpsimd.tensor_add(q12, q12, diff[:, 8, :])
                    d["q12"] = q12

                # --- B1: exp1/relu1 (fat, z1 written earlier this step) ---
                if live(i_b1):
                    d = S(i_b1)
                    e1 = elup.tile([P, 2, T], f16, tag="e1")
                    r1 = elup.tile([P, 2, T], f16, tag="r1")
                    nc.scalar.activation(e1[:], d["z1"][:], Act.Exp)
                    if RELU_ENG == "act":
                        nc.scalar.activation(r1[:], d["z1"][:], Act.Relu)
                    else:
                        nc.vector.tensor_scalar(
                            out=r1[:], in0=d["z1"][:], scalar1=0.0,
                            scalar2=None, op0=Alu.max)
                    if COMB == "std":
                        h1 = h1p.tile([P, 2, T], f16, tag="h1")
                        nc.vector.scalar_tensor_tensor(
                            out=h1[:], in0=e1[:], scalar=1.0, in1=r1[:],
                            op0=Alu.min, op1=Alu.add)
                        d["h1"] = h1
                    else:
                        m1 = h1p.tile([P, 2, T], f16, tag="m1")
                        nc.vector.tensor_scalar(
                            out=m1[:], in0=e1[:], scalar1=1.0,
                            scalar2=None, op0=Alu.min)
                        d["r1k"] = r1
                        d["m1"] = m1

                # --- S2: PE q01/q02 + ACT sqrt ---
                if live(i_s2):
                    d = S(i_s2)
                    diff = d["diff"]
                    qp = ps1.tile([P, 2, T], f32, tag="q")
                    for pi in range(2):
                        for c in range(3):
                            nc.tensor.matmul(
                                qp[:, pi, :], diag(_iWD_I()),
                                diff[:, 3 * pi + c, :],
                                start=(c == 0), stop=(c == 2))
                    dist = distp.tile([P, 3, T], f16, tag="dist")
                    nc.scalar.activation(dist[:, 0:2, :], qp[:], Act.Sqrt)
                    d["dist"] = dist

                # --- B2: L2 matmuls ---
                if live(i_b2):
                    d = S(i_b2)
                    z2 = ps2.tile([P, 2, T], f32, tag="z2")
                    for m_ in range(2):
                        nc.tensor.matmul(
                            z2[:, m_, :], diag(_iWD_B2(m_)), ones[:],
                            start=True, stop=False)
                        if COMB == "std":
                            for j in range(2):
                                nc.tensor.matmul(
                                    z2[:, m_, :], diag(_iWD_W2(m_, j)),
                                    d["h1"][:, j, :],
                                    start=False, stop=(j == 1))
                        else:
                            for j in range(2):
                                for part in (d["r1k"], d["m1"]):
                                    nc.tensor.matmul(
                                        z2[:, m_, :], diag(_iWD_W2(m_, j)),
                                        part[:, j, :], start=False,
                                        stop=(j == 1 and part is d["m1"]))
                    d["z2"] = z2

            _loop = tc.For_i(0, reps) if reps != 1 else None
            if _loop is not None:
                _loop.__enter__()

            for s in range(N_TILES + DEPTH - 1):
                step(s)

            if _loop is not None:
                _loop.__exit__(None, None, None)

    _split_sync_waits(nc, mybir, limit=1)
    return nc


def _pack_weights(W1, b1, W2, b2, W3, b3):
    W1 = np.asarray(W1, np.float32); b1 = np.asarray(b1, np.float32)
    W2 = np.asarray(W2, np.float32); b2 = np.asarray(b2, np.float32)
    W3 = np.asarray(W3, np.float32); b3 = np.asarray(b3, np.float32)
    wb = np.empty(N_WB, np.float32)
    b2a = b2 - W2.sum(axis=1)            # absorb elu(+1) shift
    b3a = b3 - W3.sum(axis=1)
    wb[0:2] = b1
    wb[2:4] = b2a
    wb[4] = b3a[0]
    wb[5] = W3[0, 0]
    wb[6] = W3[0, 1]

    eye = np.eye(P, dtype=np.float32)
    wdf = np.empty((N_WD, P, P), np.float32)
    wdf[_iWD_I()] = eye
    for k in range(2):
        for j in range(3):
            wdf[_iWD_W1(k, j)] = eye * W1[k, j]
        wdf[_iWD_B1(k)] = eye * b1[k]
    for m in range(2):
        for j in range(2):
            wdf[_iWD_W2(m, j)] = eye * W2[m, j]
        wdf[_iWD_B2(m)] = eye * b2a[m]
    return wb, wdf.astype(np.float16)


def _pack_x(x2d):
    """[n*TILE, 9] float -> [n_tiles*P, 9*T] fp16, channel-major per tile."""
    n = x2d.shape[0] // TILE
    xt = x2d.reshape(n, P, T, 9).transpose(0, 1, 3, 2)
    return np.ascontiguousarray(xt, dtype=np.float16).reshape(n * P, 9 * T)


LAST_RESULTS = None  # BassKernelResults of the most recent run (for test.py)


def kernel(X, W1, b1, W2, b2, W3, b3):
    from concourse.bass_utils import run_bass_kernel_spmd
    global LAST_RESULTS

    X = np.asarray(X, np.float32).reshape(B, 9)
    wb, wd = _pack_weights(W1, b1, W2, b2, W3, b3)

    key = (COMPUTE_DT, 1)
    if key not in _CACHE:
        _CACHE[key] = _build(COMPUTE_DT)
    nc = _CACHE[key]

    in_maps = [
        {"X": _pack_x(X[c * B_CORE:(c + 1) * B_CORE]),
         "WB": wb, "WD": wd}
        for c in range(N_CORES)
    ]
    res = run_bass_kernel_spmd(nc, in_maps, core_ids=list(range(N_CORES)))
    LAST_RESULTS = res
    out = np.concatenate(
        [res.results[c]["Y"].astype(np.float32).reshape(B_CORE)
         for c in range(N_CORES)], axis=0)
    return out.reshape(B, 1)


# revision 10
# speedup vs baseline: 1.3478x; 1.2013x over previous
# Trainium2 Bass kernel for nn_Net_4861902979707
#
# Computation (per sample, B = 4194304):
#   X [B, 3, 3] -> 3 pairwise Euclidean distances d = [d01, d02, d12]
#   h1 = elu(d @ W1.T + b1); h2 = elu(h1 @ W2.T + b2); y = h2 @ W3.T + b3
#
# Strategy: pure data parallel over 8 NeuronCores (batch split). Host does
# layout/dtype only: X is cast to fp16 and each 128xT tile is transposed to
# channel-major [128 partitions, 9 coord planes, T samples] so every on-chip
# op is a fat contiguous instruction.
#
# The per-tile computation is a long cross-engine chain; engines execute
# their streams strictly in order, so naive tile-by-tile emission runs at
# the SUM of engine times (the 140us baseline). This kernel emits a 6-deep
# software pipeline with work spread across all five engines + the DMA
# CCE ALU, so at each step every engine's next instruction depends only on
# previous-step work (or an op emitted earlier in the same step):
#   step i   : DMA in tile i
#   step i+1 : DVE diffs (3 fat subs) + squares planes 0-3;
#              GPSIMD squares planes 4-8
#   step i+2 : DMA-accumulate q12 = sq6+sq7+sq8 (SWDGE accum_op=add);
#              PE q01/q02 (identity diag, PSUM); ACT sqrt(q01,q02)
#   step i+3 : ACT sqrt(q12); PE L1 (bias via ones-plane + diag passes);
#              ACT exp1/relu1 (fat); DVE h1 = min(e1,1) + r1
#   step i+4 : PE L2; ACT exp2 + relu2[k=0]; DVE relu2[k=1]
#   step i+5 : DVE m2 = min(e2,1); PE L3 (split-feed r2/m2 + bias);
#              ACT final copy -> fp16; DMA out
# ELU identity: elu(z)+1 = relu(z) + min(exp(z), 1); the +1 shift is
# absorbed into the next layer's bias on the host (b' = b - W @ 1).
# Everything on chip is fp16 (PSUM fp32), rel err ~1e-3.
import os as _os
import numpy as np

B = 4194304
N_CORES = 8
B_CORE = B // N_CORES          # 524288
P = 128                        # partitions
T = int(_os.environ.get("TSZ", "512"))
TILE = P * T
N_TILES = B_CORE // TILE

SQ_DVE = int(_os.environ.get("SQ_DVE", "6"))   # square planes on DVE
RELU2K1 = _os.environ.get("RELU2K1", "act")    # dve | act
Q12 = _os.environ.get("Q12", "pe")            # dma | pe
DEPTH = 6

COMPUTE_DT = "fp16"

_CACHE = {}


def _split_sync_waits(nc, mybir, limit=1):
    """This walrus build rejects instructions carrying more than ~1 sem wait
    ("Too many sync wait commands"). Hoist excess waits onto NoOp carrier
    instructions (same engine, immediately before) — engine program order
    preserves the blocking semantics."""
    n_split = 0
    for f in nc.m.functions:
        for b in f.blocks:
            lst = b.instructions
            out = []
            changed = False
            for inst in lst:
                si = inst.sync_info
                if si is not None and len(si.on_wait) > limit:
                    waits = list(si.on_wait)
                    extra, keep = waits[:-limit], waits[-limit:]
                    for wi, w in enumerate(extra):
                        nop = mybir.InstNoOp(
                            name=f"wsplit-{inst.name}-{wi}")
                        nop.engine = inst.engine
                        nop.sync_info = mybir.SyncInfo(
                            on_wait=[w], on_update=[])
                        out.append(nop)
                        n_split += 1
                    inst.sync_info = type(si)(
                        on_wait=keep, on_update=list(si.on_update))
                    changed = True
                out.append(inst)
            if changed:
                b.instructions = out
    return n_split


# WD diag-matrix indices (each a [128,128] fp16 lhsT)
def _iWD_I():
    return 0
def _iWD_W1(k, j):
    return 1 + 3 * k + j
def _iWD_W2(m, j):
    return 7 + 2 * m + j
def _iWD_B1(k):
    return 11 + k
def _iWD_B2(m):
    return 13 + m
def _iWD_W3(j):
    return 15 + j
N_WD = 17
N_WB = 1  # slot 0: b3a (final-activation bias)


def _build(dt_name=None, reps=1, bench_small=False):
    import concourse.bass as bass
    import concourse.tile as tile
    import concourse.mybir as mybir

    f32 = mybir.dt.float32
    f16 = mybir.dt.float16
    Alu = mybir.AluOpType
    Act = mybir.ActivationFunctionType

    nc = bass.Bass()
    NT = 1 if bench_small else N_TILES
    X = nc.dram_tensor("X", [NT * P, 9 * T], f16, kind="ExternalInput")
    WB = nc.dram_tensor("WB", [N_WB], f32, kind="ExternalInput")
    WD = nc.dram_tensor("WD", [N_WD, P, P], f16, kind="ExternalInput")
    Y = nc.dram_tensor("Y", [NT * P, T], f16, kind="ExternalOutput")

    with tile.TileContext(nc) as tc:
        with (
            tc.tile_pool(name="singles", bufs=1) as singles,
            tc.tile_pool(name="xin", bufs=4) as xin,
            tc.tile_pool(name="diffp", bufs=3) as diffp,
            tc.tile_pool(name="distp", bufs=3) as distp,
            tc.tile_pool(name="elup", bufs=3) as elup,
            tc.tile_pool(name="h1p", bufs=3) as h1p,
            tc.tile_pool(name="outp", bufs=3) as outp,
            tc.tile_pool(name="ps1", bufs=1, space="PSUM") as ps1,
        ):
            wb = singles.tile([P, N_WB], f32)
            nc.gpsimd.dma_start(
                out=wb[:],
                in_=bass.AP(tensor=WB[:].tensor, offset=0,
                            ap=[[0, P], [1, N_WB]]))
            wd = singles.tile([P, N_WD, P], f16)
            nc.sync.dma_start(
                out=wd[:],
                in_=bass.AP(tensor=WD[:].tensor, offset=0,
                            ap=[[P, P], [P * P, N_WD], [1, P]]))
            ones = singles.tile([P, T], f16)
            nc.vector.memset(ones[:], 1.0)
            ones2 = singles.tile([P, 2, T], f16)
            nc.vector.memset(ones2[:], 1.0)

            def diag(i):  # [128,128] lhsT AP
                return wd[:, i, :]

            WIN = DEPTH + 1
            st = [dict() for _ in range(WIN)]

            def S(i):
                return st[i % WIN]

            d0 = SQ_DVE  # DVE squares planes [0, d0); GP squares [d0, 9)

            def step(s):
                i_dma, i_s1, i_s2 = s, s - 1, s - 2
                i_b1, i_b2, i_b3 = s - 3, s - 4, s - 5

                def live(i):
                    return 0 <= i < N_TILES

                # 1. DMA in (tile s)
                if live(i_dma):
                    src = 0 if bench_small else i_dma
                    xt = xin.tile([P, 9, T], f16)
                    nc.sync.dma_start(
                        out=xt[:], in_=X[src * P:(src + 1) * P, :])
                    S(i_dma)["xt"] = xt

                # 2. PE L1 (bias plane + diag passes)
                if live(i_b1):
                    d = S(i_b1)
                    z1 = ps1.tile([P, 2, T], f32, tag="z1")
                    for k in range(2):
                        nc.tensor.matmul(
                            z1[:, k, :], diag(_iWD_B1(k)), ones[:],
                            start=True, stop=False)
                        for j in range(3):
                            nc.tensor.matmul(
                                z1[:, k, :], diag(_iWD_W1(k, j)),
                                d["dist"][:, j, :],
                                start=False, stop=(j == 2))
                    d["z1"] = z1

                # 4. DVE m2 = min(e2, 1)  [e2 from last step]
                if live(i_b3):
                    d = S(i_b3)
                    m2 = outp.tile([P, 2, T], f16, tag="m2")
                    nc.vector.tensor_tensor(
                        out=m2[:], in0=d["e2"][:], in1=ones2[:],
                        op=Alu.min)
                    d["m2"] = m2

                # 5-7. DVE diffs + squares; GP squares
                if live(i_s1):
                    d = S(i_s1)
                    xt = d["xt"]
                    diff = diffp.tile([P, 9, T], f16)
                    nc.vector.tensor_sub(
                        diff[:, 6:9, :], xt[:, 3:6, :], xt[:, 6:9, :])
                    nc.gpsimd.tensor_mul(
                        diff[:, 6:9, :], diff[:, 6:9, :], diff[:, 6:9, :])
                    nc.vector.tensor_sub(
                        diff[:, 0:3, :], xt[:, 0:3, :], xt[:, 3:6, :])
                    nc.vector.tensor_sub(
                        diff[:, 3:6, :], xt[:, 0:3, :], xt[:, 6:9, :])
                    nc.vector.tensor_mul(
                        diff[:, 0:6, :], diff[:, 0:6, :], diff[:, 0:6, :])
                    d["diff"] = diff

                # 9-10. ACT exp1 / relu1 (fat, bias already in PSUM)
                if live(i_b1):
                    d = S(i_b1)
                    e1 = elup.tile([P, 2, T], f16, tag="e1")
                    r1 = elup.tile([P, 2, T], f16, tag="r1")
                    nc.scalar.activation(e1[:], d["z1"][:], Act.Exp)
                    nc.scalar.activation(r1[:], d["z1"][:], Act.Relu)
                    d["e1"], d["r1"] = e1, r1

                # 11. DVE h1 = min(e1,1) + r1
                if live(i_b1):
                    d = S(i_b1)
                    h1 = h1p.tile([P, 2, T], f16, tag="h1")
                    nc.vector.tensor_tensor(
                        out=h1[:], in0=d["e1"][:], in1=ones2[:], op=Alu.min)
                    nc.vector.tensor_add(h1[:], h1[:], d["r1"][:])
                    d["h1"] = h1

                # 12. PE q01/q02 (+ q12 if not on DMA)
                if live(i_s2):
                    d = S(i_s2)
                    diff = d["diff"]
                    qp = ps1.tile([P, 3, T], f32, tag="q")
                    for pi in range(3):
                        for c in range(3):
                            nc.tensor.matmul(
                                qp[:, pi, :], diag(_iWD_I()),
                                diff[:, 3 * pi + c, :],
                                start=(c == 0), stop=(c == 2))
                    d["qp"] = qp

                # 13. PE L2 (bias + diag passes over h1)
                if live(i_b2):
                    d = S(i_b2)
                    z2 = ps1.tile([P, 2, T], f32, tag="z2")
                    for m_ in range(2):
                        nc.tensor.matmul(
                            z2[:, m_, :], diag(_iWD_B2(m_)), ones[:],
                            start=True, stop=False)
                        for j in range(2):
                            nc.tensor.matmul(
                                z2[:, m_, :], diag(_iWD_W2(m_, j)),
                                d["h1"][:, j, :],
                                start=False, stop=(j == 1))
                    d["z2"] = z2

                # 14. PE L3 (split feed: bias + w3*(r2_j, m2_j))
                if live(i_b3):
                    d = S(i_b3)
                    yz = ps1.tile([P, T], f32, tag="yz")
                    for j in range(2):
                        nc.tensor.matmul(
                            yz[:], diag(_iWD_W3(j)), d["r2"][:, j, :],
                            start=(j == 0), stop=False)
                    for j in range(2):
                        nc.tensor.matmul(
                            yz[:], diag(_iWD_W3(j)), d["m2"][:, j, :],
                            start=False, stop=(j == 1))
                    d["yz"] = yz

                # 15. ACT sqrt(q01,q02)
                if live(i_s2):
                    d = S(i_s2)
                    dist = distp.tile([P, 3, T], f16, tag="dist")
                    nc.scalar.activation(dist[:, :, :], d["qp"][:], Act.Sqrt)
                    d["dist"] = dist

                # 16-18. elu2: ACT exp2 (fat) + relu2 split ACT/DVE
                if live(i_b2):
                    d = S(i_b2)
                    e2 = elup.tile([P, 2, T], f16, tag="e2")
                    r2 = elup.tile([P, 2, T], f16, tag="r2")
                    nc.scalar.activation(e2[:], d["z2"][:], Act.Exp)
                    if RELU2K1 == "dve":
                        nc.scalar.activation(
                            r2[:, 0, :], d["z2"][:, 0, :], Act.Relu)
                        nc.vector.tensor_scalar(
                            out=r2[:, 1, :], in0=d["z2"][:, 1, :],
                            scalar1=0.0, scalar2=None, op0=Alu.max)
                    else:
                        nc.scalar.activation(r2[:], d["z2"][:], Act.Relu)
                    
                    d["e2"], d["r2"] = e2, r2

                # 19. ACT final copy -> fp16
                if live(i_b3):
                    d = S(i_b3)
                    yt = outp.tile([P, T], f16, tag="yt")
                    nc.scalar.activation(
                        yt[:], d["yz"][:], Act.Identity,
                        bias=wb[:, 0:1], scale=1.0)
                    d["yt"] = yt

                # 20. DMA out
                if live(i_b3):
                    src = 0 if bench_small else i_b3
                    nc.sync.dma_start(
                        out=Y[src * P:(src + 1) * P, :], in_=S(i_b3)["yt"][:])

            _loop = tc.For_i(0, reps) if reps != 1 else None
            if _loop is not None:
                _loop.__enter__()

            for s in range(N_TILES + DEPTH - 1):
                step(s)

            if _loop is not None:
                _loop.__exit__(None, None, None)

    _split_sync_waits(nc, mybir, limit=1)
    return nc


def _pack_weights(W1, b1, W2, b2, W3, b3):
    W1 = np.asarray(W1, np.float32); b1 = np.asarray(b1, np.float32)
    W2 = np.asarray(W2, np.float32); b2 = np.asarray(b2, np.float32)
    W3 = np.asarray(W3, np.float32); b3 = np.asarray(b3, np.float32)
    wb = np.zeros(N_WB, np.float32)
    b2a = b2 - W2.sum(axis=1)            # absorb elu(+1) shift
    b3a = b3 - W3.sum(axis=1)
    wb[0] = b3a[0]

    eye = np.eye(P, dtype=np.float32)
    wdf = np.empty((N_WD, P, P), np.float32)
    wdf[_iWD_I()] = eye
    for k in range(2):
        for j in range(3):
            wdf[_iWD_W1(k, j)] = eye * W1[k, j]
        wdf[_iWD_B1(k)] = eye * b1[k]
    for m in range(2):
        for j in range(2):
            wdf[_iWD_W2(m, j)] = eye * W2[m, j]
        wdf[_iWD_B2(m)] = eye * b2a[m]
    for j in range(2):
        wdf[_iWD_W3(j)] = eye * W3[0, j]
    return wb, wdf.astype(np.float16)


def _pack_x(x2d):
    """[n*TILE, 9] float -> [n_tiles*P, 9*T] fp16, channel-major per tile."""
    n = x2d.shape[0] // TILE
    xt = x2d.reshape(n, P, T, 9).transpose(0, 1, 3, 2)
    return np.ascontiguousarray(xt, dtype=np.float16).reshape(n * P, 9 * T)


LAST_RESULTS = None  # BassKernelResults of the most recent run (for test.py)


def kernel(X, W1, b1, W2, b2, W3, b3):
    from concourse.bass_utils import run_bass_kernel_spmd
    global LAST_RESULTS

    X = np.asarray(X, np.float32).reshape(B, 9)
    wb, wd = _pack_weights(W1, b1, W2, b2, W3, b3)

    key = (COMPUTE_DT, 1)
    if key not in _CACHE:
        _CACHE[key] = _build(COMPUTE_DT)
    nc = _CACHE[key]

    in_maps = [
        {"X": _pack_x(X[c * B_CORE:(c + 1) * B_CORE]),
         "WB": wb, "WD": wd}
        for c in range(N_CORES)
    ]
    res = run_bass_kernel_spmd(nc, in_maps, core_ids=list(range(N_CORES)))
    LAST_RESULTS = res
    out = np.concatenate(
        [res.results[c]["Y"].astype(np.float32).reshape(B_CORE)
         for c in range(N_CORES)], axis=0)
    return out.reshape(B, 1)
